# revision 1
# baseline (speedup 1.0000x reference)
"""Trainium2 Bass kernel for top-2 MoE (B=4, T=2048, D=1024, E=8, K=2).

Strategy (expert parallelism, 8 NeuronCores):
  Launch 1 (router, data-parallel over tokens): each core takes N/8=1024
    tokens, computes logits -> softmax -> normalized top-2 gates [1024, 8]
    and the per-core sum of softmax weights [8] (for the aux loss).
  Host dispatch (the "all-to-all"): group token indices by expert, pad to a
    uniform capacity C, gather token activations in d-major (transposed)
    bf16 layout.
  Launch 2 (experts): core e holds expert e's weights (bf16, fully resident
    in SBUF) and computes yT = (relu(x @ w1 + b1) @ w2 + b2) * gate for its
    C tokens.  All matmuls are bf16 with fp32 PSUM accumulation.
  Host combine: scatter-add per-expert outputs back to token order.

kernel(**inputs) takes the full unsharded inputs and returns
(out [4, 2048, 1024] f32, aux_loss f32) exactly like the reference.
"""
from contextlib import ExitStack

import numpy as np
import ml_dtypes

import concourse.mybir as mybir
import concourse.tile as tile
from concourse import bacc
from concourse.bass_utils import run_bass_kernel_spmd

BF16 = ml_dtypes.bfloat16

B, T, D, E, K = 4, 2048, 1024, 8, 2
N = B * T                 # 8192 tokens
NT = N // 8               # tokens per core in the router launch
F = 4 * D                 # 4096
TOK = 512                 # expert-kernel token tile (matmul free dim)
DC = D // 128             # 8
FC = F // 128             # 32

_NC_CACHE = {}


def _build_router_nc():
    nc = bacc.Bacc("TRN2", target_bir_lowering=False, debug=False)
    dt = mybir.dt
    A = mybir.AluOpType
    xT = nc.dram_tensor("xT", [D, NT], dt.float32, kind="ExternalInput")
    rw = nc.dram_tensor("rw", [D, E], dt.float32, kind="ExternalInput")
    rb = nc.dram_tensor("rb", [E], dt.float32, kind="ExternalInput")
    gate = nc.dram_tensor("gate", [NT, E], dt.float32, kind="ExternalOutput")
    wsum = nc.dram_tensor("wsum", [E], dt.float32, kind="ExternalOutput")

    with tile.TileContext(nc) as tc, ExitStack() as ctx:
        const = ctx.enter_context(tc.tile_pool(name="const", bufs=1))
        xp = ctx.enter_context(tc.tile_pool(name="xp", bufs=1))
        tp = ctx.enter_context(tc.tile_pool(name="tp", bufs=4))
        sp = ctx.enter_context(tc.tile_pool(name="sp", bufs=4))
        gp = ctx.enter_context(tc.tile_pool(name="gp", bufs=3))
        pp = ctx.enter_context(tc.tile_pool(name="pp", bufs=2, space="PSUM"))
        ppi = ctx.enter_context(tc.tile_pool(name="ppi", bufs=1, space="PSUM"))

        w_sb = const.tile([128, DC, E], dt.float32)
        nc.sync.dma_start(w_sb[:], rw.ap().rearrange("(dc p) e -> p dc e", p=128))
        b_row = const.tile([1, E], dt.float32)
        nc.sync.dma_start(b_row[:], rb.ap()[None, :])
        b_bc = const.tile([128, E], dt.float32)
        nc.gpsimd.partition_broadcast(b_bc[:], b_row[0:1, :], channels=128)
        ones_sb = const.tile([128, 1], dt.float32)
        nc.vector.memset(ones_sb[:], 1.0)

        x_sb = xp.tile([128, DC * NT], dt.float32)
        for dc in range(DC):
            nc.sync.dma_start(x_sb[:, dc * NT:(dc + 1) * NT],
                              xT.ap()[dc * 128:(dc + 1) * 128, :])

        psum_i = ppi.tile([1, E], dt.float32)
        TT = NT // 128
        for tt in range(TT):
            pl = pp.tile([128, E], dt.float32)
            for dc in range(DC):
                nc.tensor.matmul(
                    pl[:],
                    x_sb[:, dc * NT + tt * 128: dc * NT + (tt + 1) * 128],
                    w_sb[:, dc, :],
                    start=(dc == 0), stop=(dc == DC - 1),
                )
            lg = tp.tile([128, E], dt.float32, tag="lg")
            nc.vector.tensor_add(lg[:], pl[:], b_bc[:])
            negm = sp.tile([128, 1], dt.float32, tag="negm")
            nc.vector.tensor_reduce(negm[:], lg[:], axis=mybir.AxisListType.X,
                                    op=A.max, negate=True)
            ex = tp.tile([128, E], dt.float32, tag="ex")
            nc.scalar.activation(ex[:], lg[:], mybir.ActivationFunctionType.Exp,
                                 bias=negm[:])
            s = sp.tile([128, 1], dt.float32, tag="s")
            nc.vector.tensor_reduce(s[:], ex[:], axis=mybir.AxisListType.X, op=A.add)
            rs = sp.tile([128, 1], dt.float32, tag="rs")
            nc.vector.reciprocal(rs[:], s[:])
            w = tp.tile([128, E], dt.float32, tag="w")
            nc.vector.tensor_scalar_mul(w[:], ex[:], rs[:])
            # importance partial: psum_i += ones.T @ w
            nc.tensor.matmul(psum_i[:], ones_sb[:], w[:],
                             start=(tt == 0), stop=(tt == TT - 1))
            # top-2 selection on ex (monotone in logits).  The softmax
            # denominator cancels in gate = w*mask/topsum = ex*mask/(m1+m2).
            m1 = sp.tile([128, 1], dt.float32, tag="m1")
            nc.vector.tensor_reduce(m1[:], ex[:], axis=mybir.AxisListType.X, op=A.max)
            ismax = tp.tile([128, E], dt.float32, tag="ismax")
            nc.vector.tensor_scalar(ismax[:], ex[:], m1[:], None, op0=A.is_equal)
            masked = tp.tile([128, E], dt.float32, tag="masked")
            nc.vector.scalar_tensor_tensor(masked[:], ismax[:], -2.0, ex[:],
                                           op0=A.mult, op1=A.mult)
            nc.vector.tensor_add(masked[:], masked[:], ex[:])
            m2 = sp.tile([128, 1], dt.float32, tag="m2")
            nc.vector.tensor_reduce(m2[:], masked[:], axis=mybir.AxisListType.X,
                                    op=A.max)
            mask2 = tp.tile([128, E], dt.float32, tag="mask2")
            nc.vector.tensor_scalar(mask2[:], ex[:], m2[:], None, op0=A.is_ge)
            nrm = sp.tile([128, 1], dt.float32, tag="nrm")
            nc.vector.tensor_add(nrm[:], m1[:], m2[:])
            rn = sp.tile([128, 1], dt.float32, tag="rn")
            nc.vector.reciprocal(rn[:], nrm[:])
            g = gp.tile([128, E], dt.float32, tag="g")
            nc.vector.scalar_tensor_tensor(g[:], ex[:], rn[:], mask2[:],
                                           op0=A.mult, op1=A.mult)
            nc.sync.dma_start(gate.ap()[tt * 128:(tt + 1) * 128, :], g[:])
        ws = gp.tile([1, E], dt.float32, tag="ws")
        nc.vector.tensor_copy(ws[:], psum_i[:])
        nc.sync.dma_start(wsum.ap()[None, :], ws[:])
    nc.compile()
    return nc


def _build_expert_nc(C: int):
    assert C % TOK == 0
    Tt = C // TOK
    nc = bacc.Bacc("TRN2", target_bir_lowering=False, debug=False)
    dt = mybir.dt
    A = mybir.AluOpType
    xT = nc.dram_tensor("xT", [D, C], dt.bfloat16, kind="ExternalInput")
    w1 = nc.dram_tensor("w1", [D, F], dt.bfloat16, kind="ExternalInput")
    w2 = nc.dram_tensor("w2", [F, D], dt.bfloat16, kind="ExternalInput")
    b1 = nc.dram_tensor("b1", [F], dt.float32, kind="ExternalInput")
    b2 = nc.dram_tensor("b2", [D], dt.float32, kind="ExternalInput")
    gate = nc.dram_tensor("gate", [C], dt.float32, kind="ExternalInput")
    yT = nc.dram_tensor("yT", [D, C], dt.float32, kind="ExternalOutput")

    with tile.TileContext(nc) as tc, ExitStack() as ctx:
        const = ctx.enter_context(tc.tile_pool(name="const", bufs=1))
        w1p = ctx.enter_context(tc.tile_pool(name="w1p", bufs=1))
        w2p = ctx.enter_context(tc.tile_pool(name="w2p", bufs=1))
        xp = ctx.enter_context(tc.tile_pool(name="xp", bufs=2))
        hp = ctx.enter_context(tc.tile_pool(name="hp", bufs=1))
        gp = ctx.enter_context(tc.tile_pool(name="gp", bufs=2))
        op = ctx.enter_context(tc.tile_pool(name="op", bufs=3))
        pp = ctx.enter_context(tc.tile_pool(name="pp", bufs=4, space="PSUM"))

        w1_sb = w1p.tile([128, DC * F], dt.bfloat16)       # [:, dc*F + f]
        for q in range(4):  # f-major-ish chunking so early f-chunks land first
            fs = F // 4
            for dc in range(DC):
                nc.sync.dma_start(
                    w1_sb[:, dc * F + q * fs: dc * F + (q + 1) * fs],
                    w1.ap()[dc * 128:(dc + 1) * 128, q * fs:(q + 1) * fs],
                )
        w2_sb = w2p.tile([128, FC * D], dt.bfloat16)       # [:, fc*D + d]
        for fc in range(FC):
            nc.sync.dma_start(w2_sb[:, fc * D:(fc + 1) * D],
                              w2.ap()[fc * 128:(fc + 1) * 128, :])
        b1_sb = const.tile([128, FC], dt.float32)
        nc.sync.dma_start(b1_sb[:], b1.ap().rearrange("(a p) -> p a", p=128))
        b2_sb = const.tile([128, DC], dt.float32)
        nc.sync.dma_start(b2_sb[:], b2.ap().rearrange("(a p) -> p a", p=128))
        gate_sb = const.tile([1, C], dt.float32)
        nc.sync.dma_start(gate_sb[:], gate.ap()[None, :])

        for t in range(Tt):
            ts = slice(t * TOK, (t + 1) * TOK)
            x_sb = xp.tile([128, DC * TOK], dt.bfloat16)   # [:, dc*TOK + tok]
            for dc in range(DC):
                nc.sync.dma_start(x_sb[:, dc * TOK:(dc + 1) * TOK],
                                  xT.ap()[dc * 128:(dc + 1) * 128, ts])
            gb = gp.tile([128, TOK], dt.float32)
            nc.gpsimd.partition_broadcast(gb[:], gate_sb[0:1, ts], channels=128)

            hT = hp.tile([128, FC * TOK], dt.bfloat16)     # [:, fc*TOK + tok]
            for fc in range(FC):
                ph = pp.tile([128, TOK], dt.float32, tag="ps")
                for dc in range(DC):
                    nc.tensor.matmul(
                        ph[:],
                        w1_sb[:, dc * F + fc * 128: dc * F + (fc + 1) * 128],
                        x_sb[:, dc * TOK:(dc + 1) * TOK],
                        start=(dc == 0), stop=(dc == DC - 1),
                    )
                nc.scalar.activation(hT[:, fc * TOK:(fc + 1) * TOK], ph[:],
                                     mybir.ActivationFunctionType.Relu,
                                     bias=b1_sb[:, fc:fc + 1])
            for dtile in range(DC):
                py = pp.tile([128, TOK], dt.float32, tag="ps")
                for fc in range(FC):
                    nc.tensor.matmul(
                        py[:],
                        w2_sb[:, fc * D + dtile * 128: fc * D + (dtile + 1) * 128],
                        hT[:, fc * TOK:(fc + 1) * TOK],
                        start=(fc == 0), stop=(fc == FC - 1),
                    )
                ysb = op.tile([128, TOK], dt.float32)
                nc.vector.scalar_tensor_tensor(
                    ysb[:], py[:], b2_sb[:, dtile:dtile + 1], gb[:],
                    op0=A.add, op1=A.mult,
                )
                nc.sync.dma_start(yT.ap()[dtile * 128:(dtile + 1) * 128, ts], ysb[:])
    nc.compile()
    return nc


def _get_nc(key, builder, *args):
    if key not in _NC_CACHE:
        _NC_CACHE[key] = builder(*args)
    return _NC_CACHE[key]


def kernel(X, router_w, router_b, w1, b1, w2, b2):
    X = np.ascontiguousarray(np.asarray(X, dtype=np.float32))
    router_w = np.ascontiguousarray(np.asarray(router_w, dtype=np.float32))
    router_b = np.ascontiguousarray(np.asarray(router_b, dtype=np.float32))
    w1 = np.asarray(w1, dtype=np.float32)
    b1 = np.asarray(b1, dtype=np.float32)
    w2 = np.asarray(w2, dtype=np.float32)
    b2 = np.asarray(b2, dtype=np.float32)

    x_flat = X.reshape(N, D)
    xT_all = np.ascontiguousarray(x_flat.T)          # [D, N] d-major

    # ---- launch 1: router (data-parallel over tokens) ----
    nc_r = _get_nc("router", _build_router_nc)
    in_maps = [
        {
            "xT": np.ascontiguousarray(xT_all[:, c * NT:(c + 1) * NT]),
            "rw": router_w,
            "rb": router_b,
        }
        for c in range(8)
    ]
    res_r = run_bass_kernel_spmd(nc_r, in_maps, list(range(8))).results

    gate_full = np.concatenate([res_r[c]["gate"] for c in range(8)], axis=0)  # [N, E]
    wsum_total = np.sum([res_r[c]["wsum"].astype(np.float64) for c in range(8)],
                        axis=0)
    importance = (wsum_total / N).astype(np.float32)
    aux_loss = np.float32(np.mean((importance - np.float32(1.0 / E)) ** 2,
                                  dtype=np.float64))

    # ---- host dispatch: group tokens by expert, pad to capacity ----
    idx = [np.flatnonzero(gate_full[:, e] > 0.0) for e in range(E)]
    counts = [len(i) for i in idx]
    C = max(512, -(-max(counts) // TOK) * TOK)

    xT_bf = xT_all.astype(BF16)
    in_maps2 = []
    for e in range(E):
        n_e = counts[e]
        xg = np.zeros((D, C), dtype=BF16)
        xg[:, :n_e] = xT_bf[:, idx[e]]
        gpad = np.zeros(C, dtype=np.float32)
        gpad[:n_e] = gate_full[idx[e], e]
        in_maps2.append({
            "xT": xg,
            "w1": w1[e].astype(BF16),
            "w2": w2[e].astype(BF16),
            "b1": np.ascontiguousarray(b1[e]),
            "b2": np.ascontiguousarray(b2[e]),
            "gate": gpad,
        })

    # ---- launch 2: expert FFN (expert parallelism) ----
    nc_e = _get_nc(("expert", C), _build_expert_nc, C)
    res_e = run_bass_kernel_spmd(nc_e, in_maps2, list(range(8))).results

    # ---- host combine: scatter-add back to token order ----
    outT = np.zeros((D, N), dtype=np.float32)
    for e in range(E):
        n_e = counts[e]
        if n_e:
            outT[:, idx[e]] += res_e[e]["yT"][:, :n_e]
    out = np.ascontiguousarray(outT.T).reshape(B, T, D)
    return out, aux_loss
